# revision 24
# baseline (speedup 1.0000x reference)
"""Trainium2 Bass kernel for nn_MultiHeadAttention_7413113553038.

Sharding: 8 cores = (batch b in {0,1}) x (query block of 512). Each core
computes all 4 heads of attention for its 512 queries against the full 2048
keys of its batch, plus the output projection, residual add and LayerNorm for
its rows. No collectives needed.

Per-core strategy (v4):
  - Host passes X_Q^T (block), X_K^T, X_V^T (pre-transposed + rolled), weights
    in fp16, and precomputed multiplicative Gaussian-bias band tables
    E = exp(bias) (neutral value 1 outside the causal band). Weights are
    packed into combined dram tensors (wqkv, e0125, tailw) so each DMA moves
    >=3KB per partition line (the DMA queues are packet-rate limited).
  - ~6 dummy matmuls at stream start keep the PE busy during the input DMA
    so the HAM clock gate lifts the 1.2 GHz cold throttle before real work.
  - Input DMAs ride 3 queues (sync/scalar HWDGE + gpsimd SWDGE) ordered by
    first use: xqt+wqkv land first, then xkt0, xvt0, bias tables, rest.
  - Q^T/K^T computed in [d, seq] layout directly (lhsT = W, rhs = X^T).
  - scores computed transposed: sT[k, q] = K Q^T (contraction d=64, heads at
    partition bases 0/64 so head pairs use distinct PE row groups).
  - p = exp(sT) with NO max-subtraction, then p *= E_slice on the 6 k-chunk
    slots covering the causal band (X_K/X_V are rolled by q0-256 so the band
    sits on static slots 0..5).
  - V is augmented with a ones-column so ctxT = V_aug.T @ p yields the
    softmax denominator Z as psum row 64 for free.
  - 1/Z: Z rows are scattered to 128 partitions with tiny PE transposes,
    reciprocal'd at [128,16], transposed back and broadcast across partitions
    with selector matmuls; ctxT is scaled before the fc projection.
  - ctxT [dm, q] is exactly the lhsT layout the fc matmul needs; LayerNorm
    via bn_stats/bn_aggr + Abs_reciprocal_sqrt (table pre-switched by a dummy
    activation right after the last exp); fc psum + residual + LN fused per
    128-row chunk in fp16; output DMA'd in halves on both HWDGE queues.
"""

import numpy as np

N_HEADS = 4
D_K = 64
B = 2
S = 2048
F = 256
QB = 512  # queries per core
P = 128
KC = S // P  # 16 k-chunks
SIGMA_HS = (5.0, 10.0, 20.0, 40.0)
LN_EPS = 1e-5
N_CORES = 8
# per-head causal-bias band width (g >= ~1e-4): ceil(4.292 * sigma)
BAND = (22, 43, 86, 172)
E01_W = 192
E25_W = 304
NWARM = 10


_CACHE = {}


def _gauss_tables():
    """Compact multiplicative Gaussian-bias band tables E = exp(g) in fp16,
    transposed-score layout (delta = q - k = off_t + j - i, off_t = 256-128t).

      e01 [4,128,192]: e01[h,i,m] = exp(g_h(m - i + 128)), slots 0,1
      e25 [4,128,304]: e25[h,i,m] = exp(g_h(m - i)), slots 2..5
    g_h(d) = exp(-d^2 / (2 sigma_h^2)) for d >= 0 else 0.
    """
    i = np.arange(P, dtype=np.float64)[None, :, None]
    sig = np.asarray(SIGMA_HS, dtype=np.float64)[:, None, None]

    m01 = np.arange(E01_W, dtype=np.float64)[None, None, :]
    d01 = m01 - i + 128.0
    g01 = np.where(d01 >= 0, np.exp(-(d01 ** 2) / (2 * sig ** 2)), 0.0)

    m25 = np.arange(E25_W, dtype=np.float64)[None, None, :]
    d25 = m25 - i
    g25 = np.where(d25 >= 0, np.exp(-(d25 ** 2) / (2 * sig ** 2)), 0.0)
    return (
        np.exp(g01).astype(np.float16),
        np.exp(g25).astype(np.float16),
    )


def _build_program():
    import concourse.bass as bass  # noqa: F401
    import concourse.tile as tile
    from concourse import bacc, mybir
    from concourse.masks import make_identity

    f32 = mybir.dt.float32
    f16 = mybir.dt.float16
    AF = mybir.ActivationFunctionType
    ALU = mybir.AluOpType

    nc = bacc.Bacc("TRN2", target_bir_lowering=False, debug=False)

    # inputs pre-packed on the host into exact SBUF layouts; small tensors
    # are fused into wide-line combined transfers
    xqt = nc.dram_tensor("xqt", [P, 2, QB], f16, kind="ExternalInput").ap()
    xkt = nc.dram_tensor("xkt", [4, P, 2, 512], f16, kind="ExternalInput").ap()
    xvt = nc.dram_tensor("xvt", [4, P, 2, 512], f16, kind="ExternalInput").ap()
    wqkv = nc.dram_tensor("wqkv", [P, 3, 2, F], f16, kind="ExternalInput").ap()
    e0125 = nc.dram_tensor(
        "e0125", [P, N_HEADS, E01_W + E25_W], f16, kind="ExternalInput"
    ).ap()
    tailw = nc.dram_tensor("tailw", [P, 6, F], f16, kind="ExternalInput").ap()
    out = nc.dram_tensor("out", [P, 4, F], f16, kind="ExternalOutput").ap()

    with tile.TileContext(nc) as tc:
        with (
            tc.tile_pool(name="wpool", bufs=1) as wpool,
            tc.tile_pool(name="xpool", bufs=1) as xpool,
            tc.tile_pool(name="proj", bufs=1) as proj,
            tc.tile_pool(name="mmps", bufs=2, space="PSUM") as mmps,
            tc.tile_pool(name="spsum", bufs=2, space="PSUM") as spsum,
            tc.tile_pool(name="cpsum", bufs=2, space="PSUM") as cpsum,
            tc.tile_pool(name="ptpool", bufs=8) as ptpool,
            tc.tile_pool(name="opool", bufs=4) as opool,
        ):
            # ---- PE warm-up: dependency-free matmuls lift the HAM cold
            # throttle (1.2 -> 2.4 GHz needs ~3.4us of sustained PE busy)
            # while the input DMAs stream in ----
            ones16 = wpool.tile([P, 512], f16, tag="ones16")
            nc.vector.memset(ones16, 1.0)
            for w in range(NWARM):
                wps = mmps.tile([P, 512], f32, tag="mm", name=f"warm{w}")
                nc.tensor.matmul(wps, ones16[:, 0:P], ones16, start=True, stop=True)

            # ---- input DMAs on 3 queues, ordered by first use; the first
            # k-block is split across both HWDGE queues and xvt rides the
            # SWDGE path gated behind wq's arrival so the bulk transfers
            # don't steal HBM bandwidth from the critical-path loads ----
            xqt_sb = xpool.tile([P, 2, QB], f16, tag="xqt")
            nc.sync.dma_start(xqt_sb, xqt)
            wqkv_sb = wpool.tile([P, 3, 2, F], f16, tag="wqkv")
            nc.scalar.dma_start(wqkv_sb[:, 0:2], wqkv[:, 0:2])
            wq_sb = wqkv_sb[:, 0]
            wk_sb = wqkv_sb[:, 1]
            wv_sb = wqkv_sb[:, 2]

            xkt_b = []
            for nb in range(4):
                kb = xpool.tile([P, 2, 512], f16, tag=f"xkt{nb}", name=f"xkt{nb}")
                if nb == 0:
                    nc.sync.dma_start(kb[:, 0:1, :], xkt[nb][:, 0:1, :])
                    nc.scalar.dma_start(kb[:, 1:2, :], xkt[nb][:, 1:2, :])
                else:
                    nc.sync.dma_start(kb, xkt[nb])
                xkt_b.append(kb)

            nc.scalar.dma_start(wqkv_sb[:, 2:3], wqkv[:, 2:3])
            e0125_sb = wpool.tile([P, N_HEADS, E01_W + E25_W], f16, tag="e0125")
            e01_sb = e0125_sb[:, :, 0:E01_W]
            e25_sb = e0125_sb[:, :, E01_W:E01_W + E25_W]

            xvt_b = []
            for nb in range(4):
                vb = xpool.tile([P, 2, 512], f16, tag=f"xvt{nb}", name=f"xvt{nb}")
                nc.gpsimd.tensor_copy(
                    vb[0:1, 0:1, 0:2], wqkv_sb[0:1, 0:1, 0:1, 0:2]
                )
                nc.gpsimd.dma_start(vb, xvt[nb])
                xvt_b.append(vb)
                if nb == 0:
                    nc.gpsimd.tensor_copy(
                        e0125_sb[0:1, 0:1, 0:2], wqkv_sb[0:1, 0:1, 0:1, 0:2]
                    )
                    nc.gpsimd.dma_start(e0125_sb, e0125)

            tailw_sb = wpool.tile([P, 6, F], f16, tag="tailw")
            nc.gpsimd.tensor_copy(
                tailw_sb[0:1, 0:1, 0:2], wqkv_sb[0:1, 0:1, 0:1, 0:2]
            )
            nc.gpsimd.dma_start(tailw_sb, tailw)
            wfc_sb = tailw_sb[:, 0:2, :]
            res_t = tailw_sb[:, 2:6, :]

            # ---- persistent tiles ----
            qt_sb = proj.tile([P, 2, QB], f16, tag="qt")
            kt_b = [
                proj.tile([P, 2, 512], f16, tag=f"kt{nb}", name=f"kt{nb}")
                for nb in range(4)
            ]
            v_b = [
                proj.tile([P, 4, N_HEADS, 65], f16, tag=f"v{nb}", name=f"v{nb}")
                for nb in range(4)
            ]
            ctx_sb = proj.tile([P, 2, QB], f16, tag="ctx")
            ztmp_z = proj.tile([P, N_HEADS, QB], f32, tag="z")
            fcacc = proj.tile([P, 4, F], f16, tag="fcacc")
            o_sb = proj.tile([P, 4, F], f16, tag="osb")

            # ---- QT projection ----
            for g in range(2):
                ps = mmps.tile([P, 512], f32, tag="mm", name=f"psq{g}")
                for c in range(2):
                    nc.tensor.matmul(
                        ps,
                        wq_sb[:, c, g * P:(g + 1) * P],
                        xqt_sb[:, c, :],
                        start=(c == 0),
                        stop=(c == 1),
                    )
                nc.vector.tensor_copy(qt_sb[:, g, :], ps)

            def project_kt(nb, groups=(0, 1)):
                for g in groups:
                    ps = mmps.tile([P, 512], f32, tag="mm", name=f"psk{nb}{g}")
                    for c in range(2):
                        nc.tensor.matmul(
                            ps,
                            wk_sb[:, c, g * P:(g + 1) * P],
                            xkt_b[nb][:, c, :],
                            start=(c == 0),
                            stop=(c == 1),
                        )
                    nc.vector.tensor_copy(kt_b[nb][:, g, :], ps)

            def project_v(nb):
                for j in range(4):
                    ps = mmps.tile([P, 512], f32, tag="mm", name=f"psv{nb}{j}")
                    psv = ps[:, :F]
                    for c in range(2):
                        nc.tensor.matmul(
                            psv,
                            xvt_b[nb][:, c, j * P:(j + 1) * P],
                            wv_sb[:, c, :],
                            start=(c == 0),
                            stop=(c == 1),
                        )
                    nc.vector.tensor_copy(
                        v_b[nb][:, j, :, 0:64],
                        psv.rearrange("p (h d) -> p h d", h=N_HEADS),
                    )
                nc.vector.tensor_copy(
                    v_b[nb][:, :, :, 64:65],
                    ones_t[:, 0:4 * N_HEADS].rearrange(
                        "p (j h one) -> p j h one", j=4, h=N_HEADS, one=1
                    ),
                )

            def attn_sc(G, kc):
                """Scores + exp for one k-chunk of head pair G; returns pt."""
                ps = spsum.tile([P, 2 * QB], f32, tag="sc", name=f"sc{G[0]}_{kc}")
                for hi, h in enumerate(G):
                    g, po = h // 2, (h % 2) * 64
                    nc.tensor.matmul(
                        ps[:, hi * QB:(hi + 1) * QB],
                        kt_b[kc // 4][po:po + 64, g, (kc % 4) * P:(kc % 4 + 1) * P],
                        qt_sb[po:po + 64, g, :],
                        start=True,
                        stop=True,
                    )
                pt = ptpool.tile([P, 2 * QB], f16, tag="pt", name=f"pt{G[0]}_{kc}")
                if G[0] == 2 and kc == KC - 1:
                    # last chunk: per-head exps so head 2's PV + Z extraction
                    # start half an exp earlier
                    nc.scalar.activation(pt[:, 0:QB], ps[:, 0:QB], AF.Exp)
                    nc.scalar.activation(pt[:, QB:], ps[:, QB:], AF.Exp)
                else:
                    nc.scalar.activation(pt, ps, AF.Exp)
                return pt

            def attn_pv(G, ctxps, kc, pt):
                """Band multiply + PV accumulate for one k-chunk."""
                for hi, h in enumerate(G):
                    if kc <= 5:
                        off_t = 256 - 128 * kc
                        j0 = max(0, -off_t)
                        j1 = min(512, BAND[h] + 128 - off_t)
                        j1 = min(512, (j1 + 7) & ~7)
                        if j1 > j0:
                            if kc <= 1:
                                c0 = (128 - 128 * kc) + j0
                                esl = e01_sb[:, h, c0:c0 + (j1 - j0)]
                            else:
                                c0 = j0 - 128 * (kc - 2)
                                esl = e25_sb[:, h, c0:c0 + (j1 - j0)]
                            nc.vector.tensor_mul(
                                pt[:, hi * QB + j0:hi * QB + j1],
                                pt[:, hi * QB + j0:hi * QB + j1],
                                esl,
                            )
                    nc.tensor.matmul(
                        ctxps[hi][0:65, :],
                        v_b[kc // 4][:, kc % 4, h, 0:65],
                        pt[:, hi * QB:(hi + 1) * QB],
                        start=(kc == 0),
                        stop=(kc == KC - 1),
                    )

            def attn_kc(G, ctxps, kc):
                attn_pv(G, ctxps, kc, attn_sc(G, kc))

            def warm_mm(name):
                """Dependency-free matmul into the (free) score psum slot to
                keep the HAM activity window busy through epilogue regions."""
                wps = spsum.tile([P, 2 * QB], f32, tag="sc", name=name)
                nc.tensor.matmul(
                    wps[:, 0:512], ones16[:, 0:P], ones16, start=True, stop=True
                )

            def warm_fill(name):
                """Small filler matmul (mm slot) to keep the PE activity
                window busy between exp-paced attention steps."""
                wps = mmps.tile([P, 512], f32, tag="mm", name=name)
                nc.tensor.matmul(
                    wps[:, 0:256], ones16[:, 0:P], ones16[:, 0:256],
                    start=True, stop=True,
                )

            def epilogue_steps(G, ctxps):
                """Per-group epilogue as emission closures so group 0's steps
                drip into group 1's instruction stream."""
                gg = G[0] // 2
                state = {}

                def s_copies():
                    # z + ctx copies split across scalar/vector so the psum
                    # slots free in half the time (z rows first: they feed
                    # the transpose chain)
                    for hi in range(2):
                        nc.scalar.copy(
                            ztmp_z[64:65, G[hi], 0:256],
                            ctxps[hi][64:65, 0:256],
                        )
                        nc.vector.tensor_copy(
                            ztmp_z[64:65, G[hi], 256:512],
                            ctxps[hi][64:65, 256:512],
                        )
                    nc.scalar.copy(ctx_sb[64:128, gg, :], ctxps[1][0:64, :])
                    nc.vector.tensor_copy(
                        ctx_sb[0:64, gg, :], ctxps[0][0:64, :]
                    )

                def ps_tile(name):
                    if gg == 0:
                        return mmps.tile([P, 512], f32, tag="mm", name=name)
                    t = spsum.tile([P, 2 * QB], f32, tag="sc", name=name)
                    return t[:, 0:512]

                def s_fwd_t():
                    zt_g = ps_tile(f"zt{gg}")
                    state["zt_g"] = zt_g
                    for hi, h in enumerate(G):
                        for qc in range(4):
                            nc.tensor.transpose(
                                zt_g[:, hi * 4 + qc:hi * 4 + qc + 1],
                                ztmp_z[64:65, h, qc * P:(qc + 1) * P],
                                ident_f[64:65, 64:65],
                            )
                    ztc = opool.tile([P, 8], f32, tag="ztc", name=f"ztc{gg}")
                    state["ztc"] = ztc
                    nc.vector.reciprocal(ztc, zt_g[:, 0:8])

                def s_back_t():
                    ztc = state["ztc"]
                    rz_ps = ps_tile(f"rz{gg}")
                    for hi in range(2):
                        nc.tensor.transpose(
                            rz_ps[0:4, hi * P:(hi + 1) * P],
                            ztc[:, hi * 4:(hi + 1) * 4],
                            ident_f,
                        )
                    rz4 = opool.tile([4, 2, P], f16, tag="rz4", name=f"rz4{gg}")
                    state["rz4"] = rz4
                    nc.vector.tensor_copy(
                        rz4, rz_ps[0:4, 0:2 * P].rearrange("p (h j) -> p h j", h=2)
                    )

                def s_zb(hi):
                    def emit():
                        h = G[hi]
                        po = (h % 2) * 64
                        zb = ps_tile(f"zb{h}")
                        for qc in range(4):
                            nc.tensor.matmul(
                                zb[:, qc * P:(qc + 1) * P],
                                sel[0:4, qc, :],
                                state["rz4"][0:4, hi, :],
                                start=True,
                                stop=True,
                            )
                        zbs = opool.tile(
                            [P, 512], f16, tag="zbs", name=f"zbs{h}"
                        )
                        nc.scalar.copy(zbs[po:po + 64, :], zb[po:po + 64, :])
                        nc.vector.tensor_mul(
                            ctx_sb[po:po + 64, gg, :],
                            ctx_sb[po:po + 64, gg, :],
                            zbs[po:po + 64, :],
                        )
                    return emit

                def s_pso():
                    """Pre-accumulate residual+group-0 fc into the pair
                    psums (off the tail critical path)."""
                    for j in range(2):
                        pso = mmps.tile(
                            [P, 512], f32, tag="mm", name=f"pso{gg}{j}"
                        )
                        state[f"pso{j}"] = pso
                        fca = fcacc[:, 2 * j:2 * j + 2, :].rearrange(
                            "p a b -> p (a b)"
                        )
                        nc.tensor.matmul(
                            pso, ident16, fca, start=True, stop=False
                        )

                def s_fc(j):
                    """fc + residual + LayerNorm for the qc pair (2j, 2j+1)."""
                    def emit():
                        if gg == 1:
                            pso = state[f"pso{j}"]
                        else:
                            pso = mmps.tile(
                                [P, 512], f32, tag="mm", name=f"pso{gg}{j}"
                            )
                        fca = fcacc[:, 2 * j:2 * j + 2, :].rearrange(
                            "p a b -> p (a b)"
                        )
                        for q in range(2):
                            nc.tensor.matmul(
                                pso[:, q * F:(q + 1) * F],
                                ctx_sb[:, gg, (2 * j + q) * P:(2 * j + q + 1) * P],
                                wfc_sb[:, gg, :],
                                start=(gg == 0),
                                stop=(gg == 0 or q == 1),
                            )
                        with nc.allow_low_precision(reason="fp16 LN tail"):
                            if gg == 0:
                                rsl = res_t[:, 2 * j:2 * j + 2, :].rearrange(
                                    "p a b -> p (a b)"
                                )
                                nc.vector.tensor_add(fca, pso, rsl)
                            else:
                                st = opool.tile(
                                    [P, 2, 6], f32, tag="st", name=f"st{j}"
                                )
                                mv = opool.tile(
                                    [P, 2, 2], f32, tag="mv", name=f"mv{j}"
                                )
                                for q in range(2):
                                    nc.vector.bn_stats(
                                        st[:, q, :], pso[:, q * F:(q + 1) * F]
                                    )
                                    nc.vector.bn_aggr(
                                        mv[:, q, :], st[:, q, :]
                                    )
                                nc.scalar.activation(
                                    mv[:, :, 1:2], mv[:, :, 1:2],
                                    AF.Abs_reciprocal_sqrt,
                                    bias=eps_t, scale=1.0,
                                )
                                for q in range(2):
                                    nc.vector.tensor_scalar(
                                        o_sb[:, 2 * j + q, :],
                                        pso[:, q * F:(q + 1) * F],
                                        mv[:, q, 0:1],
                                        mv[:, q, 1:2],
                                        op0=ALU.subtract,
                                        op1=ALU.mult,
                                    )
                                if j == 0:
                                    nc.sync.dma_start(
                                        out[:, 0:2, :], o_sb[:, 0:2, :]
                                    )
                                else:
                                    nc.scalar.dma_start(
                                        out[:, 2:4, :], o_sb[:, 2:4, :]
                                    )
                    return emit

                return [s_copies, s_fwd_t, s_back_t, s_zb(0), s_zb(1),
                        s_fc(0), s_fc(1), s_pso]

            # ---- group 0: projections interleaved with its attention.
            # kc0's scores are emitted BEFORE the V0 projection so a late
            # xvt0 DMA can't head-of-line-block the tensor queue ----
            G0, G1 = (0, 1), (2, 3)
            ctxps0 = [
                cpsum.tile([P, QB], f32, tag="ctxp", name=f"ctxp{hh}")
                for hh in G0
            ]
            project_kt(0, groups=(0,))
            pts0 = [None] * KC
            pts0[0] = attn_sc(G0, 0)
            # ---- DMA-independent setup (runs while inputs stream) ----
            ident_f = wpool.tile([P, P], f32, tag="identf")
            make_identity(nc, ident_f)
            ones_t = wpool.tile([P, P], f32, tag="ones")
            nc.vector.memset(ones_t, 1.0)
            sel = wpool.tile([4, N_HEADS, P], f16, tag="sel")
            for h in range(N_HEADS):
                nc.vector.tensor_scalar_mul(
                    sel[0:4, h, :], ones_t[0:4, :], ident_f[0:4, h:h + 1]
                )
            eps_t = wpool.tile([P, 1], f32, tag="eps")
            nc.vector.memset(eps_t, LN_EPS)
            scr_t = wpool.tile([P, 1], f32, tag="scr")
            ident16 = wpool.tile([P, P], f16, tag="ident16")
            nc.vector.tensor_copy(ident16, ident_f)

            pts0[1] = attn_sc(G0, 1)
            project_kt(0, groups=(1,))
            pts0[2] = attn_sc(G0, 2)
            project_v(0)
            for kc in range(KC):
                nb, i = kc // 4, kc % 4
                attn_pv(G0, ctxps0, kc, pts0[kc])
                if nb < 3:
                    if i == 1:
                        project_kt(nb + 1, groups=(0,))
                    elif i == 2:
                        project_kt(nb + 1, groups=(1,))
                    elif i == 3:
                        project_v(nb + 1)
                if kc + 3 < KC:
                    pts0[kc + 3] = attn_sc(G0, kc + 3)
            steps0 = epilogue_steps(G0, ctxps0)
            steps0[0]()  # z/ctx copies (scalar+vector, releases ctxps slots)

            # ---- group 1: attention with group-0 epilogue drip-fed in.
            # Scores/exp run 3 k-chunks ahead of the PV stream so drip steps
            # and band-multiply stalls never gap the exp cadence ----
            ctxps1 = [
                cpsum.tile([P, QB], f32, tag="ctxp", name=f"ctxp{hh}")
                for hh in G1
            ]
            pts1 = [None] * KC
            pts1[0] = attn_sc(G1, 0)
            pts1[1] = attn_sc(G1, 1)
            pts1[2] = attn_sc(G1, 2)
            drip = {3: steps0[1], 5: steps0[2], 7: steps0[3], 9: steps0[4],
                    11: steps0[5], 13: steps0[6]}
            for kc in range(KC):
                attn_pv(G1, ctxps1, kc, pts1[kc])
                if kc + 3 < KC:
                    pts1[kc + 3] = attn_sc(G1, kc + 3)
                if kc in drip:
                    drip[kc]()
                elif 2 <= kc <= 14 and kc % 2 == 0:
                    warm_fill(f"wf{kc}")
            steps1 = epilogue_steps(G1, ctxps1)
            steps1[0]()
            # pre-switch the ACT table set (Exp -> Abs_reciprocal_sqrt) after
            # the z copies so the ~2.7us table load overlaps the z dance
            # instead of sitting on the LN critical path (input dep on the
            # scalar z copy keeps the scheduler from hoisting it)
            nc.scalar.activation(
                scr_t[64:65, :], ztmp_z[64:65, G1[1], 0:1],
                AF.Abs_reciprocal_sqrt, bias=eps_t[64:65, :], scale=1.0,
            )
            steps1[7]()  # residual pre-accumulate into fc psums
            steps1[1]()
            warm_mm("warmt4")
            for step in steps1[2:7]:
                step()

    nc.compile()
    return nc


def get_nc():
    if "nc" not in _CACHE:
        _CACHE["nc"] = _build_program()
    return _CACHE["nc"]


def make_in_maps(input_Q, input_K, input_V, W_Q, W_K, W_V, W_fc):
    c16 = lambda a: np.ascontiguousarray(
        np.asarray(a, dtype=np.float32), dtype=np.float16
    )
    # pack [in, out]-style matrices to SBUF layout [p, c, out]
    pk_w = lambda w: c16(np.asarray(w, np.float32).reshape(2, P, -1).transpose(1, 0, 2))
    # pack an activation block X [seq, F] to X^T SBUF layout [p, c, seq]
    pk_t = lambda x: c16(np.asarray(x, np.float32).T.reshape(2, P, -1).transpose(1, 0, 2))
    # pack a rolled key/value matrix [2048, F] to per-block X^T [nb, p, c, 512]
    pk_x = lambda x: c16(
        np.asarray(x, np.float32).reshape(4, 512, 2, P).transpose(0, 3, 2, 1)
    )
    e01t, e25t = _gauss_tables()
    e01 = np.ascontiguousarray(e01t.transpose(1, 0, 2))
    e25 = np.ascontiguousarray(e25t.transpose(1, 0, 2))
    e01_neutral = np.ones_like(e01)
    wq8 = pk_w(np.asarray(W_Q, np.float32) / np.float32(np.sqrt(D_K)))
    wk = pk_w(W_K)
    wv = pk_w(W_V)
    wfc = pk_w(W_fc)
    wqkv = np.ascontiguousarray(np.stack([wq8, wk, wv], axis=1))
    e0125 = np.ascontiguousarray(np.concatenate([e01, e25], axis=2))
    e0125_neutral = np.ascontiguousarray(
        np.concatenate([e01_neutral, e25], axis=2)
    )
    in_maps = []
    for c in range(N_CORES):
        b, qb = divmod(c, 4)
        q0 = qb * QB
        r = (q0 - 256) % S
        xq_blk = np.asarray(input_Q[b][q0:q0 + QB], np.float32)
        xk_rot = np.roll(np.asarray(input_K[b], np.float32), -r, axis=0)
        xv_rot = np.roll(np.asarray(input_V[b], np.float32), -r, axis=0)
        res = c16(xq_blk.reshape(4, P, F).transpose(1, 0, 2))
        tailw = np.ascontiguousarray(np.concatenate([wfc, res], axis=1))
        in_maps.append({
            "xqt": pk_t(xq_blk),
            "xkt": pk_x(xk_rot),
            "xvt": pk_x(xv_rot),
            "wqkv": wqkv,
            "e0125": e0125_neutral if q0 == 0 else e0125,
            "tailw": tailw,
        })
    return in_maps


def kernel(input_Q, input_K, input_V, W_Q, W_K, W_V, W_fc, attn_mask=None):
    from concourse.bass_utils import run_bass_kernel_spmd

    nc = get_nc()
    in_maps = make_in_maps(input_Q, input_K, input_V, W_Q, W_K, W_V, W_fc)
    res = run_bass_kernel_spmd(nc, in_maps, core_ids=list(range(N_CORES)))
    out = np.empty((B, S, F), dtype=np.float32)
    for c in range(N_CORES):
        b, qb = divmod(c, 4)
        o = res.results[c]["out"]
        out[b, qb * QB:(qb + 1) * QB, :] = (
            o.astype(np.float32).transpose(1, 0, 2).reshape(QB, F)
        )
    return out
